# revision 1
# baseline (speedup 1.0000x reference)
"""Segment-softmax (GAT attention stage 4) Trainium2 kernel, 8 NeuronCores.

alpha_i = exp(e_i) / (sum_{j: tgt_j = tgt_i} exp(e_j) + 1e-16)

Strategy (edge-parallel, per the sharding hint):
  - Edges are sharded across the 8 cores (800k edges each).
  - Node index factorization t = r*128 + q with q in [0,128), r in [0,782).
  - Phase S (per core): bilinear one-hot histogram. For each chunk of 128
    edges (one SBUF column), DVE builds one-hot factor matrices
    (exp(e)*onehot_q) [128,128] and onehot_r [128,782] in fp16 via
    tensor_scalar (2x perf mode); the PE accumulates lhsT^T @ rhs into a
    PSUM-resident [128q, 782r] local table.
  - Table all-reduce: each core's local table goes to HBM; phase G reads all
    8 tables and sums them on-device (the 8-way all-reduce).
  - Phase G (per core): W = min(1/(T_sum+1e-16), 6e4) in fp16; per chunk the
    PE computes M = W^T-blocks @ onehot_r^T (a row-gather), DVE masks with
    onehot_q^T, and a ones-matmul reduces over q to yield the per-edge
    denominator reciprocal w; alpha = exp(e) * w.  The transposed one-hot
    row seeds come from host-permuted (pure data layout) q/r copies fed to
    K=1 outer-product matmuls.

The heavy lifting (histogram + gather) runs on device; the host only shards,
pads, permutes layouts, concatenates buffers between the two NEFF launches,
and unpads.
"""
import sys

sys.path.insert(0, "/opt/trn_rl_repo")

import numpy as np
import concourse.bacc as bacc
import concourse.mybir as mybir
import concourse.tile as tile
from concourse import bass_utils

P = 128
R = 782            # ceil(100000/128) -> node t = r*128 + q
RP = 896           # R padded to 7*128 for transposed blocks
NB_R = 7
N_CORES = 8
NUM_EDGES = 6_400_000
NUM_NODES = 100_000
EC = NUM_EDGES // N_CORES          # 800_000 edges per core
FC = EC // P                        # 6250 columns
FCP = 6256                          # padded columns: divisible by 8 and 4
B_G = 4                             # chunks per J-block, phase G (N=512)
NJB = FCP // B_G                    # 1564 J-blocks
TSLOT = (NJB + P - 1) // P          # 13 row-slots in the transposed layout
SUP = 16                            # J-blocks per select supertile (64 cols)

f16, f32 = mybir.dt.float16, mybir.dt.float32
_cache = {}


def _build_phase_s():
    nc = bacc.Bacc("TRN2", target_bir_lowering=False, debug=False,
                   enable_asserts=False)
    d_e = nc.dram_tensor("e", [P, FCP], f32, kind="ExternalInput")
    d_q = nc.dram_tensor("q", [P, FCP], f32, kind="ExternalInput")
    d_r = nc.dram_tensor("r", [P, FCP], f32, kind="ExternalInput")
    d_iota = nc.dram_tensor("iota", [P, RP], f16, kind="ExternalInput")
    d_T = nc.dram_tensor("T", [P, R], f32, kind="ExternalOutput")
    d_expe = nc.dram_tensor("expe", [P, FCP], f16, kind="ExternalOutput")
    OP = mybir.AluOpType

    with tile.TileContext(nc) as tc:
        with (
            tc.tile_pool(name="const", bufs=1) as cpool,
            tc.tile_pool(name="stage", bufs=1) as spool,
            tc.tile_pool(name="work", bufs=4) as wpool,
            tc.tile_pool(name="psum", bufs=1, space="PSUM") as ppool,
        ):
            iq = cpool.tile([P, P], f16)
            ir = cpool.tile([P, R], f16)
            nc.sync.dma_start(out=iq[:], in_=d_iota[:, 0:P])
            nc.sync.dma_start(out=ir[:], in_=d_iota[:, 0:R])

            e_sb = spool.tile([P, FCP], f32)
            q_sb = spool.tile([P, FCP], f32)
            r_sb = spool.tile([P, FCP], f32)
            expe = spool.tile([P, FCP], f32)
            expe16 = spool.tile([P, FCP], f16)
            nc.sync.dma_start(out=e_sb[:], in_=d_e[:])
            nc.sync.dma_start(out=q_sb[:], in_=d_q[:])
            nc.sync.dma_start(out=r_sb[:], in_=d_r[:])
            for c0 in range(0, FCP, 1564):
                c1 = min(c0 + 1564, FCP)
                nc.scalar.activation(expe[:, c0:c1], e_sb[:, c0:c1],
                                     mybir.ActivationFunctionType.Exp)
                nc.vector.tensor_copy(out=expe16[:, c0:c1], in_=expe[:, c0:c1])

            psumT = ppool.tile([P, R], f32, space="PSUM")
            for j in range(FCP):
                first = (j == 0)
                last = (j == FCP - 1)
                qexp_eq = wpool.tile([P, P], f16, tag="qexp_eq")
                qexp = wpool.tile([P, P], f16, tag="qexp")
                eqR = wpool.tile([P, R], f16, tag="eqR")
                nc.vector.tensor_scalar(
                    out=qexp_eq[:], in0=iq[:],
                    scalar1=q_sb[:, j:j + 1], scalar2=None,
                    op0=OP.is_equal)
                nc.scalar.activation(qexp[:], qexp_eq[:],
                                     mybir.ActivationFunctionType.Copy,
                                     scale=expe[:, j:j + 1])
                nc.vector.tensor_scalar(
                    out=eqR[:], in0=ir[:],
                    scalar1=r_sb[:, j:j + 1], scalar2=None,
                    op0=OP.is_equal)
                nc.tensor.matmul(out=psumT[:, 0:512],
                                 lhsT=qexp[:], rhs=eqR[:, 0:512],
                                 start=first, stop=last)
                nc.tensor.matmul(out=psumT[:, 512:R],
                                 lhsT=qexp[:], rhs=eqR[:, 512:R],
                                 start=first, stop=last)
            outT = spool.tile([P, R], f32)
            nc.vector.tensor_copy(out=outT[:], in_=psumT[:])
            nc.sync.dma_start(out=d_T[:], in_=outT[:])
            nc.sync.dma_start(out=d_expe[:], in_=expe16[:])
    nc.compile()
    return nc


def _build_phase_g():
    nc = bacc.Bacc("TRN2", target_bir_lowering=False, debug=False,
                   enable_asserts=False)
    d_Tall = nc.dram_tensor("Tall", [P, N_CORES, R], f32, kind="ExternalInput")
    d_qT = nc.dram_tensor("qT", [1, NJB * 512], f16, kind="ExternalInput")
    d_rT = nc.dram_tensor("rT", [1, NJB * 512], f16, kind="ExternalInput")
    d_expe = nc.dram_tensor("expe", [P, FCP], f16, kind="ExternalInput")
    d_id = nc.dram_tensor("ident", [P, P], f16, kind="ExternalInput")
    d_ones = nc.dram_tensor("ones", [P, P], f16, kind="ExternalInput")
    d_iotaPB = nc.dram_tensor("iotaPB", [P, NB_R], f32, kind="ExternalInput")
    d_alpha = nc.dram_tensor("alpha", [P, FCP], f32, kind="ExternalOutput")
    OP = mybir.AluOpType

    with tile.TileContext(nc) as tc:
        with (
            tc.tile_pool(name="const", bufs=1) as cpool,
            tc.tile_pool(name="stage", bufs=1) as spool,
            tc.tile_pool(name="work", bufs=4) as wpool,
            tc.tile_pool(name="strips", bufs=2) as stpool,
            tc.tile_pool(name="psum", bufs=2, space="PSUM") as ppool,
            tc.tile_pool(name="psumw", bufs=1, space="PSUM") as ppoolw,
        ):
            ident = cpool.tile([P, P], f16)
            ones = cpool.tile([P, P], f16)
            iPB = cpool.tile([P, NB_R], f32)
            nc.sync.dma_start(out=ident[:], in_=d_id[:])
            nc.sync.dma_start(out=ones[:], in_=d_ones[:])
            nc.sync.dma_start(out=iPB[:], in_=d_iotaPB[:])

            expe = spool.tile([P, FCP], f16)
            nc.sync.dma_start(out=expe[:], in_=d_expe[:])

            # On-device 8-way all-reduce of the local tables
            Tparts = spool.tile([P, N_CORES, R], f32)
            nc.sync.dma_start(out=Tparts[:], in_=d_Tall[:])
            Tsum = spool.tile([P, R], f32)
            nc.vector.tensor_tensor(out=Tsum[:], in0=Tparts[:, 0, :],
                                    in1=Tparts[:, 1, :], op=OP.add)
            for c in range(2, N_CORES):
                nc.vector.tensor_tensor(out=Tsum[:], in0=Tsum[:],
                                        in1=Tparts[:, c, :], op=OP.add)

            # W = min(1/(T + 1e-16), 6e4) in f16, zero-padded to RP
            W16 = spool.tile([P, RP], f16)
            Wf = spool.tile([P, R], f32)
            nc.vector.tensor_scalar_add(out=Wf[:], in0=Tsum[:], scalar1=1e-16)
            nc.vector.reciprocal(out=Wf[:], in_=Wf[:])
            nc.gpsimd.memset(W16[:], 0.0)
            nc.vector.tensor_scalar_min(out=W16[:, 0:R], in0=Wf[:], scalar1=60000.0)

            # WT_b [r', q] = W16[:, 128b:128(b+1)]^T
            WT = spool.tile([P, NB_R, P], f16)
            for b in range(NB_R):
                pt = ppoolw.tile([P, P], f16, space="PSUM", tag="psw")
                nc.tensor.transpose(out=pt[:], in_=W16[:, P * b:P * (b + 1)],
                                    identity=ident[:])
                nc.scalar.copy(out=WT[:, b, :], in_=pt[:])

            alpha_sb = spool.tile([P, FCP], f32)
            NW = B_G * P  # 512
            n_sup = (NJB + SUP - 1) // SUP
            for sup in range(n_sup):
                jb_lo = sup * SUP
                jb_hi = min(jb_lo + SUP, NJB)
                nsel = (jb_hi - jb_lo) * B_G
                psumW = ppoolw.tile([P, P], f32, space="PSUM", tag="psw")
                qstrip = stpool.tile([1, SUP * 512], f16, tag="qstrip")
                rstrip = stpool.tile([1, SUP * 512], f16, tag="rstrip")
                nc.sync.dma_start(out=qstrip[0:1, 0:nsel * P],
                                  in_=d_qT[0:1, jb_lo * 512:jb_lo * 512 + nsel * P])
                nc.sync.dma_start(out=rstrip[0:1, 0:nsel * P],
                                  in_=d_rT[0:1, jb_lo * 512:jb_lo * 512 + nsel * P])
                for jb in range(jb_lo, jb_hi):
                    j0 = jb * B_G
                    coff = (jb - jb_lo) * 512
                    # row-broadcasts via K=1 outer-product matmuls
                    p_rbc = ppool.tile([P, NW], f32, space="PSUM", tag="prbc")
                    p_qbc = ppool.tile([P, NW], f32, space="PSUM", tag="pqbc")
                    nc.tensor.matmul(
                        out=p_rbc[:], lhsT=ones[0:1, :],
                        rhs=rstrip[0:1, coff:coff + NW],
                        start=True, stop=True)
                    nc.tensor.matmul(
                        out=p_qbc[:], lhsT=ones[0:1, :],
                        rhs=qstrip[0:1, coff:coff + NW],
                        start=True, stop=True)
                    rbc = wpool.tile([P, NW], f16, tag="rbc")
                    qbc = wpool.tile([P, NW], f16, tag="qbc")
                    nc.scalar.copy(out=rbc[:], in_=p_rbc[:])
                    nc.scalar.copy(out=qbc[:], in_=p_qbc[:])
                    psumM = ppool.tile([P, NW], f32, space="PSUM", tag="psM")
                    eqRT = wpool.tile([P, NB_R, NW], f16, tag="eqRT")
                    for b in range(NB_R):
                        nc.vector.tensor_scalar(
                            out=eqRT[:, b, :], in0=rbc[:],
                            scalar1=iPB[:, b:b + 1], scalar2=None,
                            op0=OP.is_equal)
                        nc.tensor.matmul(out=psumM[:], lhsT=WT[:, b, :],
                                         rhs=eqRT[:, b, :],
                                         start=(b == 0), stop=(b == NB_R - 1))
                    Mcp = wpool.tile([P, NW], f16, tag="Mcp")
                    nc.scalar.copy(out=Mcp[:], in_=psumM[:])
                    MQ = wpool.tile([P, NW], f16, tag="MQ")
                    nc.vector.scalar_tensor_tensor(
                        out=MQ[:], in0=qbc[:], scalar=iPB[:, 0:1], in1=Mcp[:],
                        op0=OP.is_equal, op1=OP.mult)
                    for j in range(B_G):
                        col = (jb - jb_lo) * B_G + j
                        nc.tensor.matmul(out=psumW[:, col:col + 1],
                                         lhsT=MQ[:, j * P:(j + 1) * P],
                                         rhs=ones[:, 0:1], start=True, stop=True)
                c0 = jb_lo * B_G
                nc.vector.tensor_tensor(out=alpha_sb[:, c0:c0 + nsel],
                                        in0=expe[:, c0:c0 + nsel],
                                        in1=psumW[:, 0:nsel], op=OP.mult)
            nc.sync.dma_start(out=d_alpha[:], in_=alpha_sb[:])
    nc.compile()
    return nc


def _get_neffs():
    if "s" not in _cache:
        _cache["s"] = _build_phase_s()
    if "g" not in _cache:
        _cache["g"] = _build_phase_g()
    return _cache["s"], _cache["g"]


def _transposed_layout(mat16):
    """[128, FCP] f16 -> [1, NJB*512] f16 edge-major rows per J-block."""
    arr = mat16.reshape(P, NJB, B_G).transpose(1, 2, 0).reshape(1, NJB * B_G * P)
    return np.ascontiguousarray(arr)


def prep_inputs(e, edge_index):
    e = np.asarray(e, dtype=np.float32).reshape(-1)
    t = np.asarray(edge_index)[1].astype(np.int64)
    q = (t % P).astype(np.float32)
    r = (t // P).astype(np.float32)
    iota = np.arange(RP, dtype=np.float16)[None, :].repeat(P, axis=0)
    identity = np.eye(P, dtype=np.float16)
    ones_m = np.ones((P, P), dtype=np.float16)
    iotaPB = (np.arange(P, dtype=np.float32)[:, None]
              + 128.0 * np.arange(NB_R, dtype=np.float32)[None, :])
    pad = FCP - FC
    in_maps_s, qT_mats, rT_mats = [], [], []
    for c in range(N_CORES):
        sl = slice(c * EC, (c + 1) * EC)
        e_m = np.pad(e[sl].reshape(P, FC), ((0, 0), (0, pad)),
                     constant_values=-100.0)
        q_m = np.pad(q[sl].reshape(P, FC), ((0, 0), (0, pad)))
        r_m = np.pad(r[sl].reshape(P, FC), ((0, 0), (0, pad)))
        qT_mats.append(_transposed_layout(q_m.astype(np.float16)))
        rT_mats.append(_transposed_layout(r_m.astype(np.float16)))
        in_maps_s.append({"e": e_m, "q": q_m, "r": r_m, "iota": iota})
    consts = {"ident": identity, "ones": ones_m, "iotaPB": iotaPB}
    return in_maps_s, qT_mats, rT_mats, consts


def make_g_maps(res_s, qT_mats, rT_mats, consts):
    Tall = np.stack([res_s.results[c]["T"] for c in range(N_CORES)], axis=1)
    return [{"Tall": Tall, "qT": qT_mats[c], "rT": rT_mats[c],
             "expe": res_s.results[c]["expe"], **consts}
            for c in range(N_CORES)]


def kernel(e, edge_index, num_nodes):
    assert int(num_nodes) == NUM_NODES
    nc_s, nc_g = _get_neffs()
    in_maps_s, qT_mats, rT_mats, consts = prep_inputs(e, edge_index)
    res_s = bass_utils.run_bass_kernel_spmd(nc_s, in_maps_s,
                                            core_ids=list(range(N_CORES)))
    in_maps_g = make_g_maps(res_s, qT_mats, rT_mats, consts)
    res_g = bass_utils.run_bass_kernel_spmd(nc_g, in_maps_g,
                                            core_ids=list(range(N_CORES)))
    alpha = np.empty(NUM_EDGES, dtype=np.float32)
    for c in range(N_CORES):
        alpha[c * EC:(c + 1) * EC] = \
            res_g.results[c]["alpha"][:, :FC].reshape(-1)
    return alpha



# revision 2
# speedup vs baseline: 135.8190x; 135.8190x over previous
"""Segment-softmax (GAT attention stage 4) Trainium2 kernel, 8 NeuronCores.

alpha_i = exp(e_i) / (sum_{j: tgt_j = tgt_i} exp(e_j) + 1e-16)

Strategy (node-parallel sharding):
  - The host stable-sorts edges by target node (a pure data-layout
    permutation; inverted after the device run) and shards NODES across the
    8 cores (12500 nodes each).  Each node's edges are padded to D=112 slots
    (max degree for this input distribution is ~101), so every segment is a
    fixed-size contiguous run and each core owns complete segments -> no
    cross-core reduction is needed at all.
  - Per-core layout: node slot m -> partition m%128, block m//128; tile
    [128, 98 blocks, 112] fp16.  Device work per chunk of 14 blocks:
      ACT:  X = exp(E)                       (fp16 out, padding -100 -> 0)
      DVE:  S[p,b] = reduce_add(X[p,b,:])    (fp32)
      DVE:  R = 8192/(S+1e-16)               (reciprocal + scale)
      ACT/DVE (split): alpha_s[:,b,:] = X[:,b,:] * R[:,b]  (fp16, x8192
            keeps the smallest alphas in fp16 normal range)
  - Host divides by 8192 (exact power of two) while scattering back to the
    original edge order.
  All arithmetic (exp, segment sums, reciprocal, normalize) runs on device;
  the host only sorts/pads/permutes layouts.
"""
import sys

sys.path.insert(0, "/opt/trn_rl_repo")

import numpy as np
import concourse.bacc as bacc
import concourse.mybir as mybir
import concourse.tile as tile
from concourse import bass_utils

P = 128
N_CORES = 8
NUM_EDGES = 6_400_000
NUM_NODES = 100_000
NPC = NUM_NODES // N_CORES          # 12500 nodes per core
D = 112                             # padded segment length (max degree ~101)
NBLK = (NPC + P - 1) // P           # 98 node blocks per core
CH = 14                             # blocks per pipeline chunk
NCH = NBLK // CH                    # 7 chunks
SCALE = 8192.0                      # keeps alpha*SCALE in fp16 normal range

f16, f32 = mybir.dt.float16, mybir.dt.float32
_cache = {}


def _build():
    nc = bacc.Bacc("TRN2", target_bir_lowering=False, debug=False,
                   enable_asserts=False)
    d_E = nc.dram_tensor("E", [P, NBLK, D], f16, kind="ExternalInput")
    d_A = nc.dram_tensor("alpha", [P, NBLK, D], f16, kind="ExternalOutput")
    OP = mybir.AluOpType
    Exp = mybir.ActivationFunctionType.Exp
    Copy = mybir.ActivationFunctionType.Copy

    with tile.TileContext(nc) as tc:
        with (
            tc.tile_pool(name="io", bufs=3) as iopool,
            tc.tile_pool(name="sm", bufs=2) as spool,
        ):
            for k in range(NCH):
                b0 = k * CH
                E16 = iopool.tile([P, CH, D], f16, tag="E16")
                nc.sync.dma_start(out=E16[:], in_=d_E[:, b0:b0 + CH, :])
                X16 = iopool.tile([P, CH, D], f16, tag="X16")
                nc.scalar.activation(X16[:], E16[:], Exp)
                S = spool.tile([P, CH], f32, tag="S")
                nc.vector.tensor_reduce(out=S[:], in_=X16[:],
                                        axis=mybir.AxisListType.X, op=OP.add)
                R = spool.tile([P, CH], f32, tag="R")
                nc.vector.tensor_scalar_add(out=R[:], in0=S[:], scalar1=1e-16)
                nc.vector.reciprocal(out=R[:], in_=R[:])
                nc.vector.tensor_scalar_mul(out=R[:], in0=R[:], scalar1=SCALE)
                A16 = iopool.tile([P, CH, D], f16, tag="A16")
                # normalize: split the per-block scaled multiplies between the
                # scalar (ACT) and vector (DVE) engines to balance load
                for b in range(CH):
                    if b % 2 == 0:
                        nc.scalar.activation(A16[:, b, :], X16[:, b, :],
                                             Copy, scale=R[:, b:b + 1])
                    else:
                        nc.vector.tensor_scalar_mul(out=A16[:, b, :],
                                                    in0=X16[:, b, :],
                                                    scalar1=R[:, b:b + 1])
                nc.sync.dma_start(out=d_A[:, b0:b0 + CH, :], in_=A16[:])
    nc.compile()
    return nc


def _get_neff():
    if "k" not in _cache:
        _cache["k"] = _build()
    return _cache["k"]


def prep_inputs(e, edge_index):
    """Sort edges by target node, shard nodes over cores, pad segments."""
    e = np.asarray(e, dtype=np.float32).reshape(-1)
    t = np.asarray(edge_index)[1].astype(np.int64)
    perm = np.argsort(t, kind="stable")
    t_s = t[perm]
    counts = np.bincount(t, minlength=NUM_NODES)
    assert counts.max() <= D, f"max degree {counts.max()} > padded D={D}"
    starts = np.zeros(NUM_NODES + 1, dtype=np.int64)
    np.cumsum(counts, out=starts[1:])
    rank = np.arange(NUM_EDGES, dtype=np.int64) - starts[t_s]
    c = t_s // NPC
    m = t_s - c * NPC
    p = m % P
    b = m // P
    flat = ((c * P + p) * NBLK + b) * D + rank
    E = np.full(N_CORES * P * NBLK * D, -100.0, dtype=np.float16)
    E[flat] = e[perm].astype(np.float16)
    return E.reshape(N_CORES, P, NBLK, D), flat, perm


def kernel(e, edge_index, num_nodes):
    assert int(num_nodes) == NUM_NODES
    nc = _get_neff()
    E, flat, perm = prep_inputs(e, edge_index)
    in_maps = [{"E": E[c]} for c in range(N_CORES)]
    res = bass_utils.run_bass_kernel_spmd(nc, in_maps,
                                          core_ids=list(range(N_CORES)))
    A = np.stack([np.asarray(res.results[c]["alpha"])
                  for c in range(N_CORES)])
    alpha_sorted = A.reshape(-1)[flat].astype(np.float32) * np.float32(1.0 / SCALE)
    out = np.empty(NUM_EDGES, dtype=np.float32)
    out[perm] = alpha_sorted
    return out


# revision 4
# speedup vs baseline: 163.5844x; 1.2044x over previous
"""Segment-softmax (GAT attention stage 4) Trainium2 kernel, 8 NeuronCores.

alpha_i = exp(e_i) / (sum_{j: tgt_j = tgt_i} exp(e_j) + 1e-16)

Strategy (node-parallel sharding):
  - The host stable-sorts edges by target node (a pure data-layout
    permutation; inverted after the device run) and shards NODES across the
    8 cores (12500 nodes each).  Each node's edges are padded to D=112 slots
    (max degree for this input distribution is ~101), so every segment is a
    fixed-size contiguous run and each core owns complete segments -> no
    cross-core reduction is needed at all.
  - Per-core layout: node slot m -> partition m%128, block m//128; tile
    [128, 98 blocks, 112] fp16.  Device work per chunk of 14 blocks:
      ACT:  X = exp(E)                       (fp16 out, padding -100 -> 0)
      DVE:  S[p,b] = reduce_add(X[p,b,:])    (fp32)
      DVE:  R = 8192/(S+1e-16)               (reciprocal + scale)
      ACT/DVE (split): alpha_s[:,b,:] = X[:,b,:] * R[:,b]  (fp16, x8192
            keeps the smallest alphas in fp16 normal range)
  - Host divides by 8192 (exact power of two) while scattering back to the
    original edge order.
  All arithmetic (exp, segment sums, reciprocal, normalize) runs on device;
  the host only sorts/pads/permutes layouts.
"""
import sys

sys.path.insert(0, "/opt/trn_rl_repo")

import numpy as np
import concourse.bacc as bacc
import concourse.mybir as mybir
import concourse.tile as tile
from concourse import bass_utils

P = 128
N_CORES = 8
NUM_EDGES = 6_400_000
NUM_NODES = 100_000
NPC = NUM_NODES // N_CORES          # 12500 nodes per core
D = 112                             # padded segment length (max degree ~101)
NBLK = (NPC + P - 1) // P           # 98 node blocks per core
CH = 14                             # blocks per pipeline chunk
NCH = NBLK // CH                    # 7 chunks
SCALE = 8192.0                      # keeps alpha*SCALE in fp16 normal range

f16, f32 = mybir.dt.float16, mybir.dt.float32
_cache = {}


def _build():
    nc = bacc.Bacc("TRN2", target_bir_lowering=False, debug=False,
                   enable_asserts=False)
    d_E = nc.dram_tensor("E", [P, NBLK, D], f16, kind="ExternalInput")
    d_A = nc.dram_tensor("alpha", [P, NBLK, D], f16, kind="ExternalOutput")
    OP = mybir.AluOpType
    Exp = mybir.ActivationFunctionType.Exp
    Copy = mybir.ActivationFunctionType.Copy

    with tile.TileContext(nc) as tc:
        with (
            tc.tile_pool(name="io", bufs=3) as iopool,
            tc.tile_pool(name="sm", bufs=2) as spool,
        ):
            for k in range(NCH):
                b0 = k * CH
                E16 = iopool.tile([P, CH, D], f16, tag="E16")
                nc.sync.dma_start(out=E16[:], in_=d_E[:, b0:b0 + CH, :])
                X16 = iopool.tile([P, CH, D], f16, tag="X16")
                nc.scalar.activation(X16[:], E16[:], Exp)
                S = spool.tile([P, CH], f32, tag="S")
                nc.vector.tensor_reduce(out=S[:], in_=X16[:],
                                        axis=mybir.AxisListType.X, op=OP.add)
                R = spool.tile([P, CH], f32, tag="R")
                nc.vector.reciprocal(out=R[:], in_=S[:])
                # R16 = min(SCALE/S, 6e4): scaled so alpha*SCALE stays in the
                # fp16 normal range; the clamp keeps padded (S=0) slots finite
                R16 = spool.tile([P, CH], f16, tag="R16")
                nc.vector.tensor_scalar(out=R16[:], in0=R[:],
                                        scalar1=SCALE, scalar2=60000.0,
                                        op0=OP.mult, op1=OP.min)
                A16 = iopool.tile([P, CH, D], f16, tag="A16")
                # single broadcast multiply: R16 per-node scalar broadcast
                # along the D (slot) axis via a stride-0 access pattern
                Rb = R16[:].unsqueeze(2).broadcast_to([P, CH, D])
                nc.vector.tensor_tensor(out=A16[:], in0=X16[:], in1=Rb,
                                        op=OP.mult)
                nc.sync.dma_start(out=d_A[:, b0:b0 + CH, :], in_=A16[:])
    nc.compile()
    return nc


def _get_neff():
    if "k" not in _cache:
        _cache["k"] = _build()
    return _cache["k"]


def prep_inputs(e, edge_index):
    """Sort edges by target node, shard nodes over cores, pad segments."""
    e = np.asarray(e, dtype=np.float32).reshape(-1)
    t = np.asarray(edge_index)[1].astype(np.int64)
    perm = np.argsort(t, kind="stable")
    t_s = t[perm]
    counts = np.bincount(t, minlength=NUM_NODES)
    assert counts.max() <= D, f"max degree {counts.max()} > padded D={D}"
    starts = np.zeros(NUM_NODES + 1, dtype=np.int64)
    np.cumsum(counts, out=starts[1:])
    rank = np.arange(NUM_EDGES, dtype=np.int64) - starts[t_s]
    c = t_s // NPC
    m = t_s - c * NPC
    p = m % P
    b = m // P
    flat = ((c * P + p) * NBLK + b) * D + rank
    E = np.full(N_CORES * P * NBLK * D, -100.0, dtype=np.float16)
    E[flat] = e[perm].astype(np.float16)
    return E.reshape(N_CORES, P, NBLK, D), flat, perm


def kernel(e, edge_index, num_nodes):
    assert int(num_nodes) == NUM_NODES
    nc = _get_neff()
    E, flat, perm = prep_inputs(e, edge_index)
    in_maps = [{"E": E[c]} for c in range(N_CORES)]
    res = bass_utils.run_bass_kernel_spmd(nc, in_maps,
                                          core_ids=list(range(N_CORES)))
    A = np.stack([np.asarray(res.results[c]["alpha"])
                  for c in range(N_CORES)])
    alpha_sorted = A.reshape(-1)[flat].astype(np.float32) * np.float32(1.0 / SCALE)
    out = np.empty(NUM_EDGES, dtype=np.float32)
    out[perm] = alpha_sorted
    return out


# revision 5
# speedup vs baseline: 212.0282x; 1.2961x over previous
"""Segment-softmax (GAT attention stage 4) Trainium2 kernel, 8 NeuronCores.

alpha_i = exp(e_i) / (sum_{j: tgt_j = tgt_i} exp(e_j) + 1e-16)

Strategy (node-parallel sharding, degree-sorted variable-width packing):
  - The host stable-sorts edges by target node (a pure data-layout
    permutation, inverted after the device run) and shards NODES across the
    8 cores (12500 nodes each) -> each core owns complete segments, so no
    cross-core reduction is needed.
  - Within each core, nodes are ordered by degree (ascending).  Each block
    of 128 consecutive nodes (one per SBUF partition) is padded to that
    block's max degree (rounded up to 4), so the padded width tracks the
    degree distribution (~67 slots/node instead of max-degree 112).  The
    per-block widths D_b are computed from the input's degree histogram and
    shared across cores (max over cores per block index).
  - Device work per chunk of blocks:
      ACT:  X = exp(E)                      (fp16, padding -100 -> 0)
      DVE:  S[p,b] = reduce_add(X[p,b,:])   (fp32, one reduce per equal-D
                                             group via a 3D view)
      DVE:  R16 = min(8192/S, 6e4)          (fp16; x8192 keeps the smallest
                                             alphas in fp16 normal range)
      DVE:  A[:,b,:] = X[:,b,:] * R16[:,b]  (stride-0 broadcast multiply)
  - Host divides by 8192 (exact) while scattering back to original order.
  All arithmetic (exp, segment sums, reciprocal, normalize) runs on device;
  the host only sorts/pads/permutes layouts.
"""
import sys

sys.path.insert(0, "/opt/trn_rl_repo")

import numpy as np
import concourse.bacc as bacc
import concourse.mybir as mybir
import concourse.tile as tile
from concourse import bass_utils

P = 128
N_CORES = 8
NUM_EDGES = 6_400_000
NUM_NODES = 100_000
NPC = NUM_NODES // N_CORES          # 12500 nodes per core
NBLK = (NPC + P - 1) // P           # 98 node blocks per core
SCALE = 8192.0                      # keeps alpha*SCALE in fp16 normal range
TARGET_FD = 1440                    # chunk sizing for the pipeline
CAP_FD = 2048
MAXNB = 64                          # max blocks per chunk (CAP_FD/min D_b)

f16, f32 = mybir.dt.float16, mybir.dt.float32
_cache = {}


def make_schedule(counts):
    """Per-block padded widths D_b (shared across cores) from node degrees."""
    deg_sorted = np.sort(counts.reshape(N_CORES, NPC), axis=1)
    pad = NBLK * P - NPC
    deg_sorted = np.pad(deg_sorted, ((0, 0), (0, pad)))  # pad slots: degree 0
    # ascending sort + trailing zero-pad: block max = last real element;
    # simpler: blockwise max
    blockmax = deg_sorted.reshape(N_CORES, NBLK, P).max(axis=2).max(axis=0)
    Db = np.maximum(((blockmax + 3) // 4) * 4, 4).astype(np.int64)
    return tuple(int(x) for x in Db)


def make_chunks(Db):
    off = np.zeros(NBLK + 1, dtype=np.int64)
    np.cumsum(Db, out=off[1:])
    groups = []
    b = 0
    while b < NBLK:
        b2 = b
        while b2 < NBLK and Db[b2] == Db[b]:
            b2 += 1
        groups.append((b, b2 - b, Db[b]))
        b = b2
    chunks, cur, fd = [], [], 0
    for (b0, nb, Dg) in groups:
        while nb > 0:
            room = CAP_FD - fd
            if room < Dg:
                chunks.append(cur)
                cur, fd = [], 0
                continue
            take = min(nb, room // Dg)
            cur.append((b0, take, Dg))
            fd += take * Dg
            b0 += take
            nb -= take
            if fd >= TARGET_FD:
                chunks.append(cur)
                cur, fd = [], 0
    if cur:
        chunks.append(cur)
    return off, chunks


def _build(Db):
    off, chunks = make_chunks(Db)
    FD = int(off[NBLK])
    nc = bacc.Bacc("TRN2", target_bir_lowering=False, debug=False,
                   enable_asserts=False)
    d_E = nc.dram_tensor("E", [P, FD], f16, kind="ExternalInput")
    d_A = nc.dram_tensor("alpha", [P, FD], f16, kind="ExternalOutput")
    OP = mybir.AluOpType
    Exp = mybir.ActivationFunctionType.Exp

    with tile.TileContext(nc) as tc:
        with (
            tc.tile_pool(name="io", bufs=3) as iopool,
            tc.tile_pool(name="sm", bufs=3) as spool,
        ):
            for pieces in chunks:
                b_lo = pieces[0][0]
                b_hi = pieces[-1][0] + pieces[-1][1]
                o_lo, o_hi = int(off[b_lo]), int(off[b_hi])
                fdc = o_hi - o_lo
                nbc = b_hi - b_lo
                assert nbc <= MAXNB
                E16 = iopool.tile([P, CAP_FD], f16, tag="E16")
                nc.sync.dma_start(out=E16[:, 0:fdc], in_=d_E[:, o_lo:o_hi])
                X16 = iopool.tile([P, CAP_FD], f16, tag="X16")
                nc.scalar.activation(X16[:, 0:fdc], E16[:, 0:fdc], Exp)
                S = spool.tile([P, MAXNB], f32, tag="S")
                for (b0, nb, Dg) in pieces:
                    lo = int(off[b0]) - o_lo
                    v = X16[:, lo:lo + nb * Dg].rearrange(
                        "p (n d) -> p n d", d=Dg)
                    nc.vector.tensor_reduce(
                        out=S[:, b0 - b_lo:b0 - b_lo + nb], in_=v,
                        axis=mybir.AxisListType.X, op=OP.add)
                R = spool.tile([P, MAXNB], f32, tag="R")
                nc.vector.reciprocal(out=R[:, 0:nbc], in_=S[:, 0:nbc])
                R16 = spool.tile([P, MAXNB], f16, tag="R16")
                nc.vector.tensor_scalar(out=R16[:, 0:nbc], in0=R[:, 0:nbc],
                                        scalar1=SCALE, scalar2=60000.0,
                                        op0=OP.mult, op1=OP.min)
                A16 = iopool.tile([P, CAP_FD], f16, tag="A16")
                for (b0, nb, Dg) in pieces:
                    lo = int(off[b0]) - o_lo
                    xv = X16[:, lo:lo + nb * Dg].rearrange(
                        "p (n d) -> p n d", d=Dg)
                    av = A16[:, lo:lo + nb * Dg].rearrange(
                        "p (n d) -> p n d", d=Dg)
                    rb = R16[:, b0 - b_lo:b0 - b_lo + nb].unsqueeze(
                        2).broadcast_to([P, nb, Dg])
                    nc.vector.tensor_tensor(out=av, in0=xv, in1=rb,
                                            op=OP.mult)
                nc.sync.dma_start(out=d_A[:, o_lo:o_hi], in_=A16[:, 0:fdc])
    nc.compile()
    return nc


def _get_neff(Db):
    if Db not in _cache:
        _cache[Db] = _build(Db)
    return _cache[Db]


def prep_inputs(e, edge_index):
    """Sort edges by target node, degree-sort nodes, pad per-block widths."""
    e = np.asarray(e, dtype=np.float32).reshape(-1)
    t = np.asarray(edge_index)[1].astype(np.int64)
    counts = np.bincount(t, minlength=NUM_NODES)
    Db = make_schedule(counts)
    off = np.zeros(NBLK + 1, dtype=np.int64)
    np.cumsum(np.asarray(Db, dtype=np.int64), out=off[1:])
    FD = int(off[NBLK])
    # node -> rank within its core under ascending-degree order
    order = np.argsort(counts.reshape(N_CORES, NPC), axis=1, kind="stable")
    m_of = np.empty((N_CORES, NPC), dtype=np.int64)
    ar = np.arange(NPC, dtype=np.int64)
    for c in range(N_CORES):
        m_of[c, order[c]] = ar
    m = m_of.reshape(-1)                    # global node -> rank in core
    p_of = m % P
    colbase = off[m // P]                   # block start column per node
    # per-edge destination in the padded layout
    perm = np.argsort(t, kind="stable")
    t_s = t[perm]
    starts = np.zeros(NUM_NODES + 1, dtype=np.int64)
    np.cumsum(counts, out=starts[1:])
    rank = np.arange(NUM_EDGES, dtype=np.int64) - starts[t_s]
    c_e = t_s // NPC
    flat = (c_e * P + p_of[t_s]) * FD + colbase[t_s] + rank
    E = np.full(N_CORES * P * FD, -100.0, dtype=np.float16)
    E[flat] = e[perm].astype(np.float16)
    return E.reshape(N_CORES, P, FD), flat, perm, Db


def kernel(e, edge_index, num_nodes):
    assert int(num_nodes) == NUM_NODES
    E, flat, perm, Db = prep_inputs(e, edge_index)
    nc = _get_neff(Db)
    in_maps = [{"E": E[c]} for c in range(N_CORES)]
    res = bass_utils.run_bass_kernel_spmd(nc, in_maps,
                                          core_ids=list(range(N_CORES)))
    A = np.stack([np.asarray(res.results[c]["alpha"])
                  for c in range(N_CORES)])
    alpha_sorted = A.reshape(-1)[flat].astype(np.float32) * np.float32(1.0 / SCALE)
    out = np.empty(NUM_EDGES, dtype=np.float32)
    out[perm] = alpha_sorted
    return out


# revision 9
# speedup vs baseline: 236.6943x; 1.1163x over previous
"""Segment-softmax (GAT attention stage 4) Trainium2 kernel, 8 NeuronCores.

alpha_i = exp(e_i) / (sum_{j: tgt_j = tgt_i} exp(e_j) + 1e-16)

Strategy (node-parallel sharding, degree-sorted variable-width packing):
  - The host stable-sorts edges by target node (a pure data-layout
    permutation, inverted after the device run) and shards NODES across the
    8 cores (12500 nodes each) -> each core owns complete segments, so no
    cross-core reduction is needed.
  - Within each core, nodes are ordered by degree (ascending).  Each block
    of 128 consecutive nodes (one per SBUF partition) is padded to that
    block's max degree (rounded up to 4), so the padded width tracks the
    degree distribution (~67 slots/node instead of max-degree 112).  The
    per-block widths D_b are computed from the input's degree histogram and
    shared across cores (max over cores per block index).
  - Device work per chunk of blocks:
      ACT:  X = exp(E)                      (fp16, padding -100 -> 0)
      DVE:  S[p,b] = reduce_add(X[p,b,:])   (fp32, one reduce per equal-D
                                             group via a 3D view)
      DVE:  R16 = min(8192/S, 6e4)          (fp16; x8192 keeps the smallest
                                             alphas in fp16 normal range)
      DVE:  A[:,b,:] = X[:,b,:] * R16[:,b]  (stride-0 broadcast multiply)
  - Host divides by 8192 (exact) while scattering back to original order.
  All arithmetic (exp, segment sums, reciprocal, normalize) runs on device;
  the host only sorts/pads/permutes layouts.
"""
import sys

sys.path.insert(0, "/opt/trn_rl_repo")

import numpy as np
import concourse.bacc as bacc
import concourse.mybir as mybir
import concourse.tile as tile
from concourse import bass_utils

P = 128
N_CORES = 8
NUM_EDGES = 6_400_000
NUM_NODES = 100_000
NPC = NUM_NODES // N_CORES          # 12500 nodes per core
NBLK = (NPC + P - 1) // P           # 98 node blocks per core
SCALE = 8192.0                      # keeps alpha*SCALE in fp16 normal range
TARGET_FD = 1792                    # chunk sizing for the pipeline
CAP_FD = 2048
MAXNB = 64                          # max blocks per chunk (CAP_FD/min D_b)

f16, f32 = mybir.dt.float16, mybir.dt.float32
_cache = {}


def make_schedule(counts):
    """Per-block padded widths D_b (shared across cores) from node degrees."""
    deg_sorted = np.sort(counts.reshape(N_CORES, NPC), axis=1)
    pad = NBLK * P - NPC
    deg_sorted = np.pad(deg_sorted, ((0, 0), (0, pad)))  # pad slots: degree 0
    blockmax = deg_sorted.reshape(N_CORES, NBLK, P).max(axis=2).max(axis=0)
    # multiple of 8: fewer distinct widths (fewer instruction pieces) and
    # 8-byte alignment of the half-block splits used by the paired ops
    Db = np.maximum(((blockmax + 7) // 8) * 8, 8).astype(np.int64)
    return tuple(int(x) for x in Db)


def make_chunks(Db):
    off = np.zeros(NBLK + 1, dtype=np.int64)
    np.cumsum(Db, out=off[1:])
    groups = []
    b = 0
    while b < NBLK:
        b2 = b
        while b2 < NBLK and Db[b2] == Db[b]:
            b2 += 1
        groups.append((b, b2 - b, Db[b]))
        b = b2
    # chunk targets: a small first chunk (starts compute sooner) and a small
    # last chunk (final store drains sooner); big chunks in the middle
    chunks, cur, fd = [], [], 0

    def target(idx, done_fd):
        if idx == 0:
            return 512
        return TARGET_FD

    for (b0, nb, Dg) in groups:
        while nb > 0:
            room = CAP_FD - fd
            if room < Dg:
                chunks.append(cur)
                cur, fd = [], 0
                continue
            take = min(nb, room // Dg)
            tgt = target(len(chunks), fd)
            take = min(take, max(1, (tgt - fd + Dg - 1) // Dg))
            cur.append((b0, take, Dg))
            fd += take * Dg
            b0 += take
            nb -= take
            if fd >= tgt:
                chunks.append(cur)
                cur, fd = [], 0
    if cur:
        chunks.append(cur)
    # carve a small tail chunk so the last DMA-out is short
    if len(chunks) > 1:
        last = chunks[-1]
        lfd = sum(nb * Dg for (_, nb, Dg) in last)
        if lfd > 1024:
            head, tail, acc = [], [], 0
            for (b0, nb, Dg) in reversed(last):
                need = (512 - acc + Dg - 1) // Dg
                if need > 0:
                    take = min(nb, need)
                    tail.append((b0 + nb - take, take, Dg))
                    acc += take * Dg
                    nb -= take
                if nb > 0:
                    head.append((b0, nb, Dg))
            chunks[-1] = list(reversed(head))
            chunks.append(list(reversed(tail)))
    return off, chunks


def _build(Db):
    off, chunks = make_chunks(Db)
    FD = int(off[NBLK])
    nc = bacc.Bacc("TRN2", target_bir_lowering=False, debug=False,
                   enable_asserts=False)
    d_E = nc.dram_tensor("E", [P, FD], f16, kind="ExternalInput")
    d_A = nc.dram_tensor("alpha", [P, FD], f16, kind="ExternalOutput")
    OP = mybir.AluOpType
    Exp = mybir.ActivationFunctionType.Exp

    with tile.TileContext(nc) as tc:
        with (
            tc.tile_pool(name="io", bufs=3) as iopool,
            tc.tile_pool(name="sm", bufs=3) as spool,
        ):
            for pieces in chunks:
                b_lo = pieces[0][0]
                b_hi = pieces[-1][0] + pieces[-1][1]
                o_lo, o_hi = int(off[b_lo]), int(off[b_hi])
                fdc = o_hi - o_lo
                nbc = b_hi - b_lo
                assert nbc <= MAXNB
                E16 = iopool.tile([P, CAP_FD], f16, tag="E16")
                nc.sync.dma_start(out=E16[:, 0:fdc], in_=d_E[:, o_lo:o_hi])
                X16 = iopool.tile([P, CAP_FD], f16, tag="X16")
                nc.scalar.activation(X16[:, 0:fdc], E16[:, 0:fdc], Exp)
                # halving add (f16 2x mode) then reduce on half width
                H = iopool.tile([P, CAP_FD // 2], f16, tag="H")
                S = spool.tile([P, MAXNB], f32, tag="S")
                for (b0, nb, Dg) in pieces:
                    lo = int(off[b0]) - o_lo
                    hD = Dg // 2
                    v = X16[:, lo:lo + nb * Dg].rearrange(
                        "p (n d) -> p n d", d=Dg)
                    hv = H[:, lo // 2:lo // 2 + nb * hD].rearrange(
                        "p (n d) -> p n d", d=hD)
                    with nc.allow_low_precision("paired f16 half-sums"):
                        nc.vector.tensor_tensor(out=hv, in0=v[:, :, 0:hD],
                                                in1=v[:, :, hD:Dg], op=OP.add)
                    nc.vector.tensor_reduce(
                        out=S[:, b0 - b_lo:b0 - b_lo + nb], in_=hv,
                        axis=mybir.AxisListType.X, op=OP.add)
                R = spool.tile([P, MAXNB], f32, tag="R")
                nc.vector.reciprocal(out=R[:, 0:nbc], in_=S[:, 0:nbc])
                R16 = spool.tile([P, MAXNB], f16, tag="R16")
                nc.vector.tensor_scalar(out=R16[:, 0:nbc], in0=R[:, 0:nbc],
                                        scalar1=SCALE, scalar2=60000.0,
                                        op0=OP.mult, op1=OP.min)
                # duplicate each per-node scalar into adjacent pairs so the
                # broadcast multiply reads unit-stride f16 pairs (2x mode)
                R16d = spool.tile([P, 2 * MAXNB], f16, tag="R16d")
                nc.vector.tensor_copy(
                    out=R16d[:, 0:2 * nbc].rearrange("p (n t) -> p n t", t=2),
                    in_=R16[:, 0:nbc].unsqueeze(2).broadcast_to([P, nbc, 2]))
                A16 = iopool.tile([P, CAP_FD], f16, tag="A16")
                for (b0, nb, Dg) in pieces:
                    lo = int(off[b0]) - o_lo
                    lb = b0 - b_lo
                    hD = Dg // 2
                    xv = X16[:, lo:lo + nb * Dg].rearrange(
                        "p (n h t) -> p n h t", h=hD, t=2)
                    av = A16[:, lo:lo + nb * Dg].rearrange(
                        "p (n h t) -> p n h t", h=hD, t=2)
                    rb = R16d[:, 2 * lb:2 * (lb + nb)].rearrange(
                        "p (n t) -> p n t", t=2).unsqueeze(2).broadcast_to(
                        [P, nb, hD, 2])
                    nc.vector.tensor_tensor(out=av, in0=xv, in1=rb,
                                            op=OP.mult)
                nc.sync.dma_start(out=d_A[:, o_lo:o_hi], in_=A16[:, 0:fdc])
    nc.compile()
    return nc


def _get_neff(Db):
    if Db not in _cache:
        _cache[Db] = _build(Db)
    return _cache[Db]


def prep_inputs(e, edge_index):
    """Sort edges by target node, degree-sort nodes, pad per-block widths."""
    e = np.asarray(e, dtype=np.float32).reshape(-1)
    t = np.asarray(edge_index)[1].astype(np.int64)
    counts = np.bincount(t, minlength=NUM_NODES)
    Db = make_schedule(counts)
    off = np.zeros(NBLK + 1, dtype=np.int64)
    np.cumsum(np.asarray(Db, dtype=np.int64), out=off[1:])
    FD = int(off[NBLK])
    # node -> rank within its core under ascending-degree order
    order = np.argsort(counts.reshape(N_CORES, NPC), axis=1, kind="stable")
    m_of = np.empty((N_CORES, NPC), dtype=np.int64)
    ar = np.arange(NPC, dtype=np.int64)
    for c in range(N_CORES):
        m_of[c, order[c]] = ar
    m = m_of.reshape(-1)                    # global node -> rank in core
    p_of = m % P
    colbase = off[m // P]                   # block start column per node
    # per-edge destination in the padded layout
    perm = np.argsort(t, kind="stable")
    t_s = t[perm]
    starts = np.zeros(NUM_NODES + 1, dtype=np.int64)
    np.cumsum(counts, out=starts[1:])
    rank = np.arange(NUM_EDGES, dtype=np.int64) - starts[t_s]
    c_e = t_s // NPC
    flat = (c_e * P + p_of[t_s]) * FD + colbase[t_s] + rank
    E = np.full(N_CORES * P * FD, -100.0, dtype=np.float16)
    E[flat] = e[perm].astype(np.float16)
    return E.reshape(N_CORES, P, FD), flat, perm, Db


def kernel(e, edge_index, num_nodes):
    assert int(num_nodes) == NUM_NODES
    E, flat, perm, Db = prep_inputs(e, edge_index)
    nc = _get_neff(Db)
    in_maps = [{"E": E[c]} for c in range(N_CORES)]
    res = bass_utils.run_bass_kernel_spmd(nc, in_maps,
                                          core_ids=list(range(N_CORES)))
    A = np.stack([np.asarray(res.results[c]["alpha"])
                  for c in range(N_CORES)])
    alpha_sorted = A.reshape(-1)[flat].astype(np.float32) * np.float32(1.0 / SCALE)
    out = np.empty(NUM_EDGES, dtype=np.float32)
    out[perm] = alpha_sorted
    return out


# revision 12
# speedup vs baseline: 238.7694x; 1.0088x over previous
"""Segment-softmax (GAT attention stage 4) Trainium2 kernel, 8 NeuronCores.

alpha_i = exp(e_i) / (sum_{j: tgt_j = tgt_i} exp(e_j) + 1e-16)

Strategy (node-parallel sharding, degree-sorted variable-width packing):
  - The host stable-sorts edges by target node (a pure data-layout
    permutation, inverted after the device run) and shards NODES across the
    8 cores (12500 nodes each) -> each core owns complete segments, so no
    cross-core reduction is needed.
  - Within each core, nodes are ordered by degree (ascending).  Each block
    of 128 consecutive nodes (one per SBUF partition) is padded to that
    block's max degree (rounded up to 4), so the padded width tracks the
    degree distribution (~67 slots/node instead of max-degree 112).  The
    per-block widths D_b are computed from the input's degree histogram and
    shared across cores (max over cores per block index).
  - Device work per chunk of blocks:
      ACT:  X = exp(E)                      (fp16, padding -100 -> 0)
      DVE:  S[p,b] = reduce_add(X[p,b,:])   (fp32, one reduce per equal-D
                                             group via a 3D view)
      DVE:  R16 = min(8192/S, 6e4)          (fp16; x8192 keeps the smallest
                                             alphas in fp16 normal range)
      DVE:  A[:,b,:] = X[:,b,:] * R16[:,b]  (stride-0 broadcast multiply)
  - Host divides by 8192 (exact) while scattering back to original order.
  All arithmetic (exp, segment sums, reciprocal, normalize) runs on device;
  the host only sorts/pads/permutes layouts.
"""
import sys

sys.path.insert(0, "/opt/trn_rl_repo")

import numpy as np
import concourse.bacc as bacc
import concourse.mybir as mybir
import concourse.tile as tile
from concourse import bass_utils

P = 128
N_CORES = 8
NUM_EDGES = 6_400_000
NUM_NODES = 100_000
NPC = NUM_NODES // N_CORES          # 12500 nodes per core
NBLK = (NPC + P - 1) // P           # 98 node blocks per core
SCALE = 8192.0                      # keeps alpha*SCALE in fp16 normal range
TARGET_FD = 1792                    # chunk sizing for the pipeline
CAP_FD = 2048
MAXNB = 64                          # max blocks per chunk (CAP_FD/min D_b)

f16, f32 = mybir.dt.float16, mybir.dt.float32
_cache = {}


def make_schedule(counts):
    """Per-block padded widths D_b (shared across cores) from node degrees."""
    deg_sorted = np.sort(counts.reshape(N_CORES, NPC), axis=1)
    pad = NBLK * P - NPC
    deg_sorted = np.pad(deg_sorted, ((0, 0), (0, pad)))  # pad slots: degree 0
    blockmax = deg_sorted.reshape(N_CORES, NBLK, P).max(axis=2).max(axis=0)
    # multiple of 8: fewer distinct widths (fewer instruction pieces) and
    # 8-byte alignment of the half-block splits used by the paired ops
    Db = np.maximum(((blockmax + 7) // 8) * 8, 8).astype(np.int64)
    return tuple(int(x) for x in Db)


def make_chunks(Db):
    off = np.zeros(NBLK + 1, dtype=np.int64)
    np.cumsum(Db, out=off[1:])
    groups = []
    b = 0
    while b < NBLK:
        b2 = b
        while b2 < NBLK and Db[b2] == Db[b]:
            b2 += 1
        groups.append((b, b2 - b, Db[b]))
        b = b2
    # chunk targets: a small first chunk (starts compute sooner) and a small
    # last chunk (final store drains sooner); big chunks in the middle
    chunks, cur, fd = [], [], 0

    def target(idx, done_fd):
        if idx == 0:
            return 512
        return TARGET_FD

    for (b0, nb, Dg) in groups:
        while nb > 0:
            room = CAP_FD - fd
            if room < Dg:
                chunks.append(cur)
                cur, fd = [], 0
                continue
            take = min(nb, room // Dg)
            tgt = target(len(chunks), fd)
            take = min(take, max(1, (tgt - fd + Dg - 1) // Dg))
            cur.append((b0, take, Dg))
            fd += take * Dg
            b0 += take
            nb -= take
            if fd >= tgt:
                chunks.append(cur)
                cur, fd = [], 0
    if cur:
        chunks.append(cur)
    # carve a small tail chunk so the last DMA-out is short
    if len(chunks) > 1:
        last = chunks[-1]
        lfd = sum(nb * Dg for (_, nb, Dg) in last)
        if lfd > 1024:
            head, tail, acc = [], [], 0
            for (b0, nb, Dg) in reversed(last):
                need = (512 - acc + Dg - 1) // Dg
                if need > 0:
                    take = min(nb, need)
                    tail.append((b0 + nb - take, take, Dg))
                    acc += take * Dg
                    nb -= take
                if nb > 0:
                    head.append((b0, nb, Dg))
            chunks[-1] = list(reversed(head))
            chunks.append(list(reversed(tail)))
    return off, chunks


def _build(Db):
    off, chunks = make_chunks(Db)
    FD = int(off[NBLK])
    nc = bacc.Bacc("TRN2", target_bir_lowering=False, debug=False,
                   enable_asserts=False)
    d_E = nc.dram_tensor("E", [P, FD], f16, kind="ExternalInput")
    d_A = nc.dram_tensor("alpha", [P, FD], f16, kind="ExternalOutput")
    OP = mybir.AluOpType
    Exp = mybir.ActivationFunctionType.Exp

    with tile.TileContext(nc) as tc:
        with (
            tc.tile_pool(name="io", bufs=4) as iopool,
            tc.tile_pool(name="sm", bufs=4) as spool,
        ):
            for ci, pieces in enumerate(chunks):
                b_lo = pieces[0][0]
                b_hi = pieces[-1][0] + pieces[-1][1]
                o_lo, o_hi = int(off[b_lo]), int(off[b_hi])
                fdc = o_hi - o_lo
                nbc = b_hi - b_lo
                assert nbc <= MAXNB
                E16 = iopool.tile([P, CAP_FD], f16, tag="E16")
                nc.sync.dma_start(out=E16[:, 0:fdc], in_=d_E[:, o_lo:o_hi])
                X16 = iopool.tile([P, CAP_FD], f16, tag="X16")
                nc.scalar.activation(X16[:, 0:fdc], E16[:, 0:fdc], Exp)
                # halving add (f16 2x mode) then reduce on half width
                H = iopool.tile([P, CAP_FD // 2], f16, tag="H")
                S = spool.tile([P, MAXNB], f32, tag="S")
                for (b0, nb, Dg) in pieces:
                    lo = int(off[b0]) - o_lo
                    hD = Dg // 2
                    v = X16[:, lo:lo + nb * Dg].rearrange(
                        "p (n d) -> p n d", d=Dg)
                    hv = H[:, lo // 2:lo // 2 + nb * hD].rearrange(
                        "p (n d) -> p n d", d=hD)
                    with nc.allow_low_precision("paired f16 half-sums"):
                        nc.vector.tensor_tensor(out=hv, in0=v[:, :, 0:hD],
                                                in1=v[:, :, hD:Dg], op=OP.add)
                    nc.vector.tensor_reduce(
                        out=S[:, b0 - b_lo:b0 - b_lo + nb], in_=hv,
                        axis=mybir.AxisListType.X, op=OP.add)
                R = spool.tile([P, MAXNB], f32, tag="R")
                nc.vector.reciprocal_approx_fast(out=R[:, 0:nbc],
                                                 in_=S[:, 0:nbc])
                R16 = spool.tile([P, MAXNB], f16, tag="R16")
                nc.vector.tensor_scalar(out=R16[:, 0:nbc], in0=R[:, 0:nbc],
                                        scalar1=SCALE, scalar2=60000.0,
                                        op0=OP.mult, op1=OP.min)
                # duplicate each per-node scalar into adjacent pairs so the
                # broadcast multiply reads unit-stride f16 pairs (2x mode)
                R16d = spool.tile([P, 2 * MAXNB], f16, tag="R16d")
                nc.vector.tensor_copy(
                    out=R16d[:, 0:2 * nbc].rearrange("p (n t) -> p n t", t=2),
                    in_=R16[:, 0:nbc].unsqueeze(2).broadcast_to([P, nbc, 2]))
                A16 = iopool.tile([P, CAP_FD], f16, tag="A16")
                for (b0, nb, Dg) in pieces:
                    lo = int(off[b0]) - o_lo
                    lb = b0 - b_lo
                    hD = Dg // 2
                    xv = X16[:, lo:lo + nb * Dg].rearrange(
                        "p (n h t) -> p n h t", h=hD, t=2)
                    av = A16[:, lo:lo + nb * Dg].rearrange(
                        "p (n h t) -> p n h t", h=hD, t=2)
                    rb = R16d[:, 2 * lb:2 * (lb + nb)].rearrange(
                        "p (n t) -> p n t", t=2).unsqueeze(2).broadcast_to(
                        [P, nb, hD, 2])
                    nc.vector.tensor_tensor(out=av, in0=xv, in1=rb,
                                            op=OP.mult)
                # stores go out on the gpsimd (SWDGE) ring so loads keep the
                # sync HWDGE ring to themselves; the final (small) store uses
                # the lower-latency HWDGE path since it gates kernel end
                if ci == len(chunks) - 1:
                    nc.sync.dma_start(out=d_A[:, o_lo:o_hi], in_=A16[:, 0:fdc])
                else:
                    nc.gpsimd.dma_start(out=d_A[:, o_lo:o_hi],
                                        in_=A16[:, 0:fdc])
    nc.compile()
    return nc


def _get_neff(Db):
    if Db not in _cache:
        _cache[Db] = _build(Db)
    return _cache[Db]


def prep_inputs(e, edge_index):
    """Sort edges by target node, degree-sort nodes, pad per-block widths."""
    e = np.asarray(e, dtype=np.float32).reshape(-1)
    t = np.asarray(edge_index)[1].astype(np.int64)
    counts = np.bincount(t, minlength=NUM_NODES)
    Db = make_schedule(counts)
    off = np.zeros(NBLK + 1, dtype=np.int64)
    np.cumsum(np.asarray(Db, dtype=np.int64), out=off[1:])
    FD = int(off[NBLK])
    # node -> rank within its core under ascending-degree order
    order = np.argsort(counts.reshape(N_CORES, NPC), axis=1, kind="stable")
    m_of = np.empty((N_CORES, NPC), dtype=np.int64)
    ar = np.arange(NPC, dtype=np.int64)
    for c in range(N_CORES):
        m_of[c, order[c]] = ar
    m = m_of.reshape(-1)                    # global node -> rank in core
    p_of = m % P
    colbase = off[m // P]                   # block start column per node
    # per-edge destination in the padded layout
    perm = np.argsort(t, kind="stable")
    t_s = t[perm]
    starts = np.zeros(NUM_NODES + 1, dtype=np.int64)
    np.cumsum(counts, out=starts[1:])
    rank = np.arange(NUM_EDGES, dtype=np.int64) - starts[t_s]
    c_e = t_s // NPC
    flat = (c_e * P + p_of[t_s]) * FD + colbase[t_s] + rank
    E = np.full(N_CORES * P * FD, -100.0, dtype=np.float16)
    E[flat] = e[perm].astype(np.float16)
    return E.reshape(N_CORES, P, FD), flat, perm, Db


def kernel(e, edge_index, num_nodes):
    assert int(num_nodes) == NUM_NODES
    E, flat, perm, Db = prep_inputs(e, edge_index)
    nc = _get_neff(Db)
    in_maps = [{"E": E[c]} for c in range(N_CORES)]
    res = bass_utils.run_bass_kernel_spmd(nc, in_maps,
                                          core_ids=list(range(N_CORES)))
    A = np.stack([np.asarray(res.results[c]["alpha"])
                  for c in range(N_CORES)])
    alpha_sorted = A.reshape(-1)[flat].astype(np.float32) * np.float32(1.0 / SCALE)
    out = np.empty(NUM_EDGES, dtype=np.float32)
    out[perm] = alpha_sorted
    return out
